# revision 1
# baseline (speedup 1.0000x reference)
"""Block-local self-attention (BLOCK=128, 3-block windows + global token) on 8
Trainium2 NeuronCores.

Sharding: batch*heads = 32 (n,h) pairs -> 4 pairs per core, no cross-core comms.

Per-core device kernel, per pair:
  - scoresT slabs: for each k-block j (32 of them), one matmul computes
    scoresT[k in block j, q in blocks qlo..qlo+2] + a q0 column, with the
    additive mask folded in as a 65th contraction row (K-side row = mask,
    Q-side row = 1.0) and the 1/sqrt(d) scale folded into Q on the host.
  - exp on ScalarE (batched 2 slabs/op, PSUM->SBUF bf16).
  - PV: ctx[q,d] accumulated in PSUM over the 3 contributing slabs with the
    exp tile as the stationary operand; a 65th V column of ones accumulates
    the softmax denominator in the same matmuls.
  - global slot: every window also attends to token 0's K/V.  e0[q] =
    exp(q.k0*scale + m0) is computed as 32 tiny matmuls into a [128,32]
    PSUM column tile, exp'd, flattened to row layout by an SBUF->SBUF DMA,
    and added to each window as a K=1 rank-1 matmul (V'[0] row).
  - global query row: each slab's q0 column is exp'd with the rest of the
    slab; 32 accumulating [1,65] matmuls against V' give softmax(q0.K) @ V.
  - normalize: DVE reciprocal of the denominator column + tensor_scalar mul.

Output is written in a (pair, mgroup, partition, window, d) layout so every
DMA descriptor row is >= 1KB; the host inverts the layout.
"""

import numpy as np
import ml_dtypes

N, H, T, D = 2, 16, 4000, 64
BLOCK = 128
TP = 4096            # padded token count (32 blocks)
W = 32               # number of 128-blocks
NCORES = 8
PAIRS = N * H        # 32
PPC = PAIRS // NCORES  # pairs per core
SLABW = 3 * BLOCK + 1  # 385: 3 q-blocks + q0 column
NEG = -30000.0
SCALE = 1.0 / np.sqrt(np.float32(D))

_prog_cache = {}


def _qlo(j):
    return min(max(j - 1, 0), W - 3)


def _build_program():
    if "nc" in _prog_cache:
        return _prog_cache["nc"]

    import concourse.bacc as bacc
    import concourse.mybir as mybir
    from concourse import tile

    dt = mybir.dt
    EXP = mybir.ActivationFunctionType.Exp

    nc = bacc.Bacc("TRN2", target_bir_lowering=False, debug=False,
                   num_devices=NCORES)
    qts_d = nc.dram_tensor("qts", [PPC, 65, W * SLABW], dt.bfloat16,
                           kind="ExternalInput").ap()
    kte_d = nc.dram_tensor("kte", [PPC, 65, TP], dt.bfloat16,
                           kind="ExternalInput").ap()
    vp_d = nc.dram_tensor("vp", [PPC, 128, W * 65], dt.bfloat16,
                          kind="ExternalInput").ap()
    v0sel_d = nc.dram_tensor("v0sel", [PPC, W, W * 65], dt.bfloat16,
                             kind="ExternalInput").ap()
    out_d = nc.dram_tensor("out", [PPC, 8, 128, 256], dt.float32,
                           kind="ExternalOutput").ap()

    with tile.TileContext(nc) as tc:
        with (
            tc.tile_pool(name="qts", bufs=3) as qts_pool,
            tc.tile_pool(name="kte", bufs=3) as kte_pool,
            tc.tile_pool(name="vp", bufs=3) as vp_pool,
            tc.tile_pool(name="ex", bufs=4) as ex_pool,
            tc.tile_pool(name="small", bufs=3) as small_pool,
            tc.tile_pool(name="outp", bufs=3) as out_pool,
            tc.tile_pool(name="sc", bufs=2, space="PSUM") as sc_pool,
            tc.tile_pool(name="ctx", bufs=3, space="PSUM") as ctx_pool,
            tc.tile_pool(name="aux", bufs=1, space="PSUM") as aux_pool,
        ):
            def load_pair(p):
                kte_t = kte_pool.tile([65, TP], dt.bfloat16, tag="kte",
                                      name=f"kte_{p}")
                nc.sync.dma_start(kte_t[:], kte_d[p])
                qts_t = qts_pool.tile([65, W * SLABW], dt.bfloat16, tag="qts",
                                      name=f"qts_{p}")
                nc.sync.dma_start(qts_t[:], qts_d[p])
                vp_t = vp_pool.tile([128, W * 65], dt.bfloat16, tag="vp",
                                    name=f"vp_{p}")
                nc.sync.dma_start(vp_t[:], vp_d[p])
                v0sel_t = vp_pool.tile([W, W * 65], dt.bfloat16, tag="v0sel",
                                       name=f"v0sel_{p}")
                nc.sync.dma_start(v0sel_t[:], v0sel_d[p])
                return qts_t, kte_t, vp_t, v0sel_t

            # PE warm-up: ~56 dense N=512 matmuls on memset data keep the
            # array busy (and un-throttle the HAM clock gate to 2.4 GHz)
            # while the first pair's inputs stream in.
            warm_sb = small_pool.tile([128, 1024], dt.bfloat16, tag="warm")
            nc.gpsimd.memset(warm_sb[:], 0.25)
            warm_ps = sc_pool.tile([128, 512], dt.float32, tag="sc",
                                   name="warm_ps")
            for r in range(64):
                nc.tensor.matmul(warm_ps[:], warm_sb[:, 0:128],
                                 warm_sb[:, 0:512], start=True, stop=True)

            pending = {0: load_pair(0)}
            for p in range(PPC):
                qts_t, kte_t, vp_t, v0sel_t = pending.pop(p)

                def qblock(i, qts_t=qts_t):
                    # QT block i as a [65, 128] slice of the slab-packed tile
                    if i <= W - 3:
                        s, g = i + 1, i - _qlo(i + 1)
                    else:
                        s, g = W - 1, i - _qlo(W - 1)
                    base = s * SLABW + g * 128
                    return qts_t[:, base:base + 128]

                # ---- e0: token-0 key/value slot scores for every q --------
                s0_ps = aux_pool.tile([128, W], dt.float32, tag="aux")
                for i in range(W):
                    nc.tensor.matmul(s0_ps[:, i:i + 1], qblock(i),
                                     kte_t[:, 0:1], start=True, stop=True)
                e0_sb = small_pool.tile([128, 128], dt.bfloat16, tag="e0")
                nc.gpsimd.memset(e0_sb[:, W:128], 0.0)
                nc.scalar.activation(e0_sb[:, 0:W], s0_ps[:], EXP)
                # transpose via the DMA xbar: e0T[i, q-in-block] on rows 0..31.
                # Issued on the Activation HWDGE ring so it is not queued
                # behind the next pair's bulk input loads (the SP ring is
                # FIFO, and the rank-1 weight loads block on this transpose).
                e0T = small_pool.tile([128, 128], dt.bfloat16, tag="e0T")
                nc.scalar.dma_start_transpose(e0T[:], e0_sb[:])

                # prefetch the next pair's inputs (emitted after the e0T
                # transpose so the SP DMA ring serves this pair first)
                if p + 1 < PPC:
                    pending[p + 1] = load_pair(p + 1)

                gctx_ps = aux_pool.tile([1, 65], dt.float32, tag="aux")

                ex_tiles = {}
                out_tiles = {}

                def emit_windows(ws, p=p, vp_t=vp_t, e0T=e0T, v0sel_t=v0sel_t,
                                 ex_tiles=ex_tiles, out_tiles=out_tiles):
                    # interleave the windows' accumulation chains so
                    # consecutive PE matmuls hit different PSUM banks
                    # (same-bank chains serialize the weight loads)
                    cts, seqs = {}, {}
                    for w in ws:
                        cts[w] = ctx_pool.tile([128, 65], dt.float32,
                                               tag="ctx", name=f"ct_{p}_{w}")
                        seq = []
                        slabs = [s for s in (w - 1, w, w + 1) if 0 <= s < W]
                        for idx, s in enumerate(slabs):
                            g = w - _qlo(s)
                            exm = ex_tiles[s // 2]
                            base = (s % 2) * SLABW + g * 128
                            seq.append((exm[:, base:base + 128],
                                        vp_t[:, s * 65:(s + 1) * 65],
                                        idx == 0, False))
                        # global slot: += e0[q] (x) V'[token 0], via the
                        # one-hot v0sel operand (row w = V'[0], else zero)
                        seq.append((e0T[0:W, :],
                                    v0sel_t[:, w * 65:(w + 1) * 65],
                                    False, True))
                        seqs[w] = seq
                    for r in range(max(len(s) for s in seqs.values())):
                        for w in ws:
                            if r < len(seqs[w]):
                                lhsT, rhs, st, sp = seqs[w][r]
                                nc.tensor.matmul(cts[w][:], lhsT, rhs,
                                                 start=st, stop=sp)
                    for w in ws:
                        ct = cts[w]
                        rc = small_pool.tile([128, 1], dt.float32, tag="rc",
                                             name=f"rc_{p}_{w}")
                        nc.vector.reciprocal_approx_fast(rc[:], ct[:, 64:65])
                        mi, wi = w // 4, w % 4
                        if wi == 0:
                            out_tiles[mi] = out_pool.tile(
                                [128, 256], dt.float32, tag="out",
                                name=f"out_{p}_{mi}")
                        ot = out_tiles[mi]
                        nc.vector.tensor_scalar_mul(
                            ot[:, wi * 64:(wi + 1) * 64], ct[:, 0:64], rc[:])
                        if wi == 3:
                            nc.sync.dma_start(out_d[p, mi], ot[:])

                def emit_qk(m):
                    sc = sc_pool.tile([128, 1024], dt.float32, tag="sc",
                                      name=f"sc_{p}_{m}")
                    for h2 in range(2):
                        j = 2 * m + h2
                        nc.tensor.matmul(
                            sc[:, h2 * 512:h2 * 512 + SLABW],
                            kte_t[:, j * 128:(j + 1) * 128],
                            qts_t[:, j * SLABW:(j + 1) * SLABW],
                            start=True, stop=True)
                    return sc

                def emit_exp(m, sc):
                    ex = ex_pool.tile([128, 2 * SLABW], dt.bfloat16, tag="ex",
                                      name=f"ex_{p}_{m}")
                    nc.scalar.activation(
                        ex[:].rearrange("p (b x) -> p b x", x=SLABW),
                        sc[:].rearrange("p (b x) -> p b x", x=512)[:, :, 0:SLABW],
                        EXP)
                    ex_tiles[m] = ex
                    if m == 0:
                        # token 0 is served by the global slot; zero its
                        # window-path row (q0 column kept for the global row)
                        nc.gpsimd.memset(ex[0:1, 0:3 * BLOCK], 0.0)

                def emit_batch_consume(m):
                    ex = ex_tiles[m]
                    for h2 in range(2):
                        j = 2 * m + h2
                        nc.tensor.matmul(
                            gctx_ps[:],
                            ex[:, h2 * SLABW + 384:h2 * SLABW + 385],
                            vp_t[:, j * 65:(j + 1) * 65],
                            start=(j == 0), stop=(j == W - 1))
                    ws = []
                    if m > 0:
                        ws.append(2 * m - 1)
                    ws.append(2 * m)
                    if m == W // 2 - 1:
                        ws.append(W - 1)
                    emit_windows(ws)

                # two-deep software pipeline: QK two batches ahead and
                # exp one batch ahead of the PV/gctx consumption, so the
                # PE never waits on a just-issued ACT exp.
                scs = {0: emit_qk(0), 1: emit_qk(1)}
                for m in range(W // 2):
                    emit_exp(m, scs.pop(m))
                    if m + 2 < W // 2:
                        scs[m + 2] = emit_qk(m + 2)
                    if m >= 1:
                        emit_batch_consume(m - 1)
                emit_batch_consume(W // 2 - 1)

                # global query row -> overwrites token 0's output
                rg = small_pool.tile([1, 1], dt.float32, tag="rg")
                nc.vector.reciprocal_approx_fast(rg[:], gctx_ps[0:1, 64:65])
                go = small_pool.tile([1, 64], dt.float32, tag="go")
                nc.vector.tensor_scalar_mul(go[:], gctx_ps[0:1, 0:64], rg[:])
                nc.sync.dma_start(out_d[p, 0, 0:1, 0:64], go[:])

    nc.compile()
    _prog_cache["nc"] = nc
    return nc


def _prep_core_inputs(q, k, v, mask):
    """q,k,v: (PAIRS, T, D) f32; mask: (N, T) f32.  Returns list of per-core
    input dicts (bf16 device layouts)."""
    bf16 = ml_dtypes.bfloat16
    in_maps = []
    for c in range(NCORES):
        qts = np.zeros((PPC, 65, W * SLABW), np.float32)
        kte = np.zeros((PPC, 65, TP), np.float32)
        vp = np.zeros((PPC, 128, W * 65), np.float32)
        v0sel = np.zeros((PPC, W, W * 65), np.float32)
        for pp in range(PPC):
            pair = c * PPC + pp
            n = pair // H
            m_n = mask[n]
            # QT_ext: [65, TP], rows 0..63 = scale * Q^T, row 64 = 1.0
            QT = np.zeros((65, TP), np.float32)
            QT[:D, :T] = q[pair].T * SCALE
            QT[D, :] = 1.0
            # KT_ext: rows 0..63 = K^T, row 64 = additive mask vector
            KT = np.zeros((65, TP), np.float32)
            KT[:D, :T] = k[pair].T
            KT[D, :T] = m_n
            KT[D, T:] = NEG
            KT[D, 0] = m_n[0]  # token 0 served via the global slot
            kte[pp] = KT
            for j in range(W):
                lo = _qlo(j)
                qts[pp, :, j * SLABW:j * SLABW + 3 * BLOCK] = \
                    QT[:, lo * 128:(lo + 3) * 128]
                qts[pp, :, j * SLABW + 3 * BLOCK] = QT[:, 0]
            # V': (TP, 65) = [V | ones] -> (128, W, 65)
            Vp = np.zeros((TP, 65), np.float32)
            Vp[:T, :D] = v[pair]
            Vp[:, D] = 1.0
            Vp[T:, D] = 1.0  # pad rows get exp=0 anyway; keep denom harmless
            vp[pp] = Vp.reshape(W, 128, 65).transpose(1, 0, 2).reshape(128, W * 65)
            for i in range(W):
                v0sel[pp, i, i * 65:(i + 1) * 65] = Vp[0]
        in_maps.append({
            "qts": qts.astype(bf16),
            "kte": kte.astype(bf16),
            "vp": vp.astype(bf16),
            "v0sel": v0sel.astype(bf16),
        })
    return in_maps


def _unshard(results):
    out = np.empty((PAIRS, T, D), np.float32)
    for c in range(NCORES):
        o = results[c]["out"]  # (PPC, 8, 128, 256)
        o = o.reshape(PPC, 8, 128, 4, 64).transpose(0, 1, 3, 2, 4)
        o = o.reshape(PPC, TP, D)[:, :T, :]
        out[c * PPC:(c + 1) * PPC] = o
    return out.reshape(N, H, T, D)


def _run(inputs, trace=False, tmpdir=None):
    from concourse.bass_utils import run_bass_kernel_spmd

    q = np.asarray(inputs["query_layer"], np.float32).reshape(PAIRS, T, D)
    k = np.asarray(inputs["key_layer"], np.float32).reshape(PAIRS, T, D)
    v = np.asarray(inputs["value_layer"], np.float32).reshape(PAIRS, T, D)
    mask = np.asarray(inputs["attention_mask"], np.float32).reshape(N, T)

    nc = _build_program()
    in_maps = _prep_core_inputs(q, k, v, mask)
    res = run_bass_kernel_spmd(nc, in_maps, list(range(NCORES)),
                               trace=trace, tmpdir=tmpdir)
    return _unshard(res.results), res


def kernel(query_layer, key_layer, value_layer, attention_mask):
    out, _ = _run({
        "query_layer": query_layer,
        "key_layer": key_layer,
        "value_layer": value_layer,
        "attention_mask": attention_mask,
    })
    return out



# revision 2
# speedup vs baseline: 1.4494x; 1.4494x over previous
"""Block-local self-attention (BLOCK=128, 3-block windows + global token) on 8
Trainium2 NeuronCores.

Sharding: batch*heads = 32 (n,h) pairs -> 4 pairs per core, no cross-core comms.

Per-core device kernel, per pair (all heavy O(T*window) work):
  - QK: for each k-block j (32), one matmul scoresT[k in j, q in blocks
    qlo..qlo+2] = K_j^T Q (stationary = K_j [65,128] incl. a mask row,
    moving = a contiguous [65,384] slice of the natural Q^T layout; the
    1/sqrt(d) scale is folded into Q on the host, the additive mask rides
    as a 65th contraction row).  3 slabs share one [128,1536] PSUM tile.
  - exp on ScalarE: one ACTIVATE per 3-slab batch, PSUM->SBUF bf16.
  - PV transposed: stationary = V'_j [128,65] ([V | ones] block; the ones
    column accumulates the softmax denominator), moving = 128-wide exp
    slices -> ctxT[d, q] accumulated in PSUM.  4 windows share one PSUM
    bank ([65,512]), pre-cleared by a rank-1 zero matmul (start=True
    clears has_written for the whole bank, so per-window start flags
    cannot be used in a shared bank).
  - DVE copies each closed ctxT bank to an SBUF out tile; 2 DMAs/pair.

Host side (O(T*D) only): input packing, the global-token rank-1 slot
(e0 = exp(q . k0)), the global query row (token 0 attends to all keys),
and the final division by the denominator row.
"""

import numpy as np
import ml_dtypes

N, H, T, D = 2, 16, 4000, 64
BLOCK = 128
TP = 4096            # padded token count (32 blocks)
W = 32               # number of 128-blocks
NCORES = 8
PAIRS = N * H        # 32
PPC = PAIRS // NCORES  # pairs per core
NEG = -30000.0
SCALE = 1.0 / np.sqrt(np.float32(D))
BQ = 3               # slabs (k-blocks) per QK/exp batch

_prog_cache = {}


def _qlo(j):
    return min(max(j - 1, 0), W - 3)


def _batches():
    out, j = [], 0
    while j < W:
        out.append(list(range(j, min(j + BQ, W))))
        j += BQ
    return out


def _contributors(w):
    # slabs feeding window w, in increasing slab order, with the ex-slice g
    return [(j, w - _qlo(j)) for j in (w - 1, w, w + 1) if 0 <= j < W]


def _build_program():
    if "nc" in _prog_cache:
        return _prog_cache["nc"]

    import concourse.bacc as bacc
    import concourse.mybir as mybir
    from concourse import tile

    dt = mybir.dt
    EXP = mybir.ActivationFunctionType.Exp

    nc = bacc.Bacc("TRN2", target_bir_lowering=False, debug=False,
                   num_devices=NCORES)
    qts_d = nc.dram_tensor("qts", [PPC, 65, TP], dt.bfloat16,
                           kind="ExternalInput").ap()
    kte_d = nc.dram_tensor("kte", [PPC, 65, TP], dt.bfloat16,
                           kind="ExternalInput").ap()
    vp_d = nc.dram_tensor("vp", [PPC, 128, W * 65], dt.bfloat16,
                          kind="ExternalInput").ap()
    out_d = nc.dram_tensor("out", [PPC, 65, TP], dt.float32,
                           kind="ExternalOutput").ap()

    batches = _batches()
    NB = len(batches)

    with tile.TileContext(nc) as tc:
        with (
            tc.tile_pool(name="qts", bufs=2) as qts_pool,
            tc.tile_pool(name="kte", bufs=2) as kte_pool,
            tc.tile_pool(name="vp", bufs=2) as vp_pool,
            tc.tile_pool(name="ex", bufs=3) as ex_pool,
            tc.tile_pool(name="small", bufs=1) as small_pool,
            tc.tile_pool(name="outp", bufs=2) as out_pool,
            tc.tile_pool(name="sc", bufs=2, space="PSUM") as sc_pool,
            tc.tile_pool(name="ctx", bufs=2, space="PSUM") as ctx_pool,
        ):
            def load_pair(p):
                kte_t = kte_pool.tile([65, TP], dt.bfloat16, tag="kte",
                                      name=f"kte_{p}")
                nc.sync.dma_start(kte_t[:], kte_d[p])
                qts_t = qts_pool.tile([65, TP], dt.bfloat16, tag="qts",
                                      name=f"qts_{p}")
                nc.sync.dma_start(qts_t[:], qts_d[p])
                vp_t = vp_pool.tile([128, W * 65], dt.bfloat16, tag="vp",
                                    name=f"vp_{p}")
                nc.sync.dma_start(vp_t[:], vp_d[p])
                return qts_t, kte_t, vp_t

            # zero source for the PSUM bank-clear matmuls
            zero_sb = small_pool.tile([1, 576], dt.bfloat16, tag="zero")
            nc.gpsimd.memset(zero_sb[:], 0.0)

            # PE warm-up: dense matmuls on memset data un-throttle the HAM
            # clock gate while the first pair's inputs stream in.
            warm_sb = small_pool.tile([128, 512], dt.bfloat16, tag="warm")
            nc.gpsimd.memset(warm_sb[:], 0.25)
            warm_ps = sc_pool.tile([128, 512], dt.float32, tag="sc",
                                   name="warm_ps")
            for r in range(12):
                nc.tensor.matmul(warm_ps[:], warm_sb[:, 0:128],
                                 warm_sb[:], start=True, stop=True)

            pending = {0: load_pair(0)}
            for p in range(PPC):
                qts_t, kte_t, vp_t = pending.pop(p)
                if p + 1 < PPC:
                    pending[p + 1] = load_pair(p + 1)

                outt = out_pool.tile([65, TP], dt.float32, tag="out",
                                     name=f"out_{p}")

                ex_tiles = {}
                ctx_tiles = {}
                slab_batch = {}
                for bi, sl in enumerate(batches):
                    for j in sl:
                        slab_batch[j] = bi

                def emit_qk(b, qts_t=qts_t, kte_t=kte_t, p=p):
                    sl = batches[b]
                    sc = sc_pool.tile([128, BQ * 512], dt.float32, tag="sc",
                                      name=f"sc_{p}_{b}")
                    for i, j in enumerate(sl):
                        c0 = _qlo(j) * 128
                        nc.tensor.matmul(
                            sc[:, i * 512:i * 512 + 384],
                            kte_t[:, j * 128:(j + 1) * 128],
                            qts_t[:, c0:c0 + 384],
                            start=True, stop=True)
                    return sc

                def emit_exp(b, sc, p=p):
                    nb = len(batches[b])
                    ex = ex_pool.tile([128, BQ * 384], dt.bfloat16, tag="ex",
                                      name=f"ex_{p}_{b}")
                    nc.scalar.activation(
                        ex[:, 0:nb * 384].rearrange("p (b x) -> p b x", x=384),
                        sc[:].rearrange("p (b x) -> p b x", x=512)[:, 0:nb, 0:384],
                        EXP)
                    ex_tiles[b] = ex

                def touch_group(g, p=p):
                    # allocate + bank-clear the PSUM accumulator for windows
                    # 4g..4g+3 (one full bank; all window matmuls then
                    # accumulate with start=False)
                    ct = ctx_pool.tile([65, 512], dt.float32, tag="ctx",
                                       name=f"ct_{p}_{g}")
                    nc.tensor.matmul(ct[:], zero_sb[0:1, 0:65],
                                     zero_sb[0:1, 64:576],
                                     start=True, stop=False,
                                     skip_group_check=True)
                    ctx_tiles[g] = ct
                    return ct

                def emit_pv(b, vp_t=vp_t, p=p, outt=outt):
                    for j in batches[b]:
                        # windows fed by slab j
                        for w in (j - 1, j, j + 1):
                            if not (0 <= w < W):
                                continue
                            g, wi = w // 4, w % 4
                            if g not in ctx_tiles:
                                touch_group(g)
                            ct = ctx_tiles[g]
                            gsl = w - _qlo(j)
                            bb = slab_batch[j]
                            exm = ex_tiles[bb]
                            off = (j - batches[bb][0]) * 384 + gsl * 128
                            jlast = min(w + 1, W - 1)
                            nc.tensor.matmul(
                                ct[:, wi * 128:(wi + 1) * 128],
                                vp_t[:, j * 65:(j + 1) * 65],
                                exm[:, off:off + 128],
                                start=False, stop=(j == jlast),
                                skip_group_check=True)
                        # close groups whose last contributor is slab j
                        for g in list(ctx_tiles):
                            if min(4 * g + 4, W - 1) == j:
                                ct = ctx_tiles.pop(g)
                                nc.vector.tensor_scalar_mul(
                                    outt[:, g * 512:(g + 1) * 512],
                                    ct[:], 1.0)
                                if g == 3:
                                    nc.sync.dma_start(
                                        out_d[p, :, 0:2048], outt[:, 0:2048])
                                elif g == 7:
                                    nc.sync.dma_start(
                                        out_d[p, :, 2048:4096],
                                        outt[:, 2048:4096])

                # software pipeline: QK two batches ahead, exp one ahead of
                # the PV consumption; PV before the next QK so the PE never
                # queues behind an exp it doesn't depend on.
                scs = {0: emit_qk(0), 1: emit_qk(1)}
                for b in range(NB):
                    emit_exp(b, scs.pop(b))
                    if b >= 1:
                        emit_pv(b - 1)
                    if b + 2 < NB:
                        scs[b + 2] = emit_qk(b + 2)
                emit_pv(NB - 1)

    nc.compile()
    _prog_cache["nc"] = nc
    return nc


def _prep_core_inputs(q, k, v, mask):
    """q,k,v: (PAIRS, T, D) f32; mask: (N, T) f32.  Returns per-core input
    dicts (bf16 device layouts)."""
    bf16 = ml_dtypes.bfloat16
    mpair = np.repeat(mask, H, axis=0)              # (PAIRS, T)

    qt = np.zeros((PAIRS, 65, TP), np.float32)
    qt[:, :D, :T] = q.transpose(0, 2, 1) * SCALE
    qt[:, D, :] = 1.0

    kt = np.zeros((PAIRS, 65, TP), np.float32)
    kt[:, :D, :T] = k.transpose(0, 2, 1)
    kt[:, D, :T] = mpair
    kt[:, D, T:] = NEG
    kt[:, D, 0] = NEG                               # k0 served by global slot

    vp = np.zeros((PAIRS, TP, 65), np.float32)
    vp[:, :T, :D] = v
    vp[:, :, D] = 1.0
    vp = vp.reshape(PAIRS, W, 128, 65).transpose(0, 2, 1, 3) \
           .reshape(PAIRS, 128, W * 65)

    qt = qt.astype(bf16)
    kt = kt.astype(bf16)
    vp = vp.astype(bf16)
    in_maps = []
    for c in range(NCORES):
        s = slice(c * PPC, (c + 1) * PPC)
        in_maps.append({"qts": qt[s], "kte": kt[s], "vp": vp[s]})
    return in_maps


def _host_global(q, k, v, mask):
    """e0 (token-0 key slot, per query) and the global query row, in f32."""
    mpair = np.repeat(mask, H, axis=0)              # (PAIRS, T)
    k0 = k[:, 0, :]                                 # (PAIRS, D)
    s0 = np.einsum('ptd,pd->pt', q, k0) * SCALE + mpair[:, 0:1]
    e0 = np.exp(s0)                                 # (PAIRS, T)

    q0 = q[:, 0, :]                                 # (PAIRS, D)
    gs = np.einsum('pd,ptd->pt', q0, k) * SCALE + mpair
    gs -= gs.max(axis=1, keepdims=True)
    ge = np.exp(gs)
    grow = np.einsum('pt,ptd->pd', ge, v) / ge.sum(axis=1, keepdims=True)
    return e0, grow


def _unshard(results, e0, grow, v0):
    o = np.concatenate([r["out"] for r in results], axis=0)  # (PAIRS,65,TP)
    ctx = o[:, :D, :T]                              # (PAIRS, D, T)
    den = o[:, D, :T] + e0                          # (PAIRS, T)
    ctx = ctx + e0[:, None, :] * v0[:, :, None]
    out = (ctx / den[:, None, :]).transpose(0, 2, 1)  # (PAIRS, T, D)
    out[:, 0, :] = grow
    return np.ascontiguousarray(out.reshape(N, H, T, D), dtype=np.float32)


def _run(inputs, trace=False, tmpdir=None):
    from concourse.bass_utils import run_bass_kernel_spmd

    q = np.asarray(inputs["query_layer"], np.float32).reshape(PAIRS, T, D)
    k = np.asarray(inputs["key_layer"], np.float32).reshape(PAIRS, T, D)
    v = np.asarray(inputs["value_layer"], np.float32).reshape(PAIRS, T, D)
    mask = np.asarray(inputs["attention_mask"], np.float32).reshape(N, T)

    nc = _build_program()
    in_maps = _prep_core_inputs(q, k, v, mask)
    e0, grow = _host_global(q, k, v, mask)
    res = run_bass_kernel_spmd(nc, in_maps, list(range(NCORES)),
                               trace=trace, tmpdir=tmpdir)
    return _unshard(res.results, e0, grow, v[:, 0, :]), res


def kernel(query_layer, key_layer, value_layer, attention_mask):
    out, _ = _run({
        "query_layer": query_layer,
        "key_layer": key_layer,
        "value_layer": value_layer,
        "attention_mask": attention_mask,
    })
    return out


# revision 4
# speedup vs baseline: 1.7364x; 1.1980x over previous
"""Block-local self-attention (BLOCK=128, 3-block windows + global token) on 8
Trainium2 NeuronCores.

Sharding: batch*heads = 32 (n,h) pairs -> 4 pairs per core, no cross-core comms.

Per-core device kernel, per pair (all heavy O(T*window) work):
  - QK: for each k-block j (32), one matmul scoresT[k in j, q in blocks
    qlo..qlo+2] = K_j^T Q (stationary = K_j [65,128] incl. a mask row,
    moving = a contiguous [65,384] slice of the natural Q^T layout; the
    1/sqrt(d) scale is folded into Q on the host, the additive mask rides
    as a 65th contraction row).  3 slabs share one [128,1536] PSUM tile.
  - exp on ScalarE: one ACTIVATE per 3-slab batch, PSUM->SBUF bf16.
  - PV transposed: stationary = V'_j [128,65] ([V | ones] block; the ones
    column accumulates the softmax denominator), moving = 128-wide exp
    slices -> ctxT[d, q] accumulated in PSUM.  4 windows share one PSUM
    bank ([65,512]); window 4g's first matmul opens the bank with
    start=True (the whole-bank has_written clear happens before any other
    window touches the bank, and later windows' first writes land on
    cleared bits = overwrite), so no separate bank-clear is needed.
  - DVE copies each closed ctxT bank to an SBUF out tile; 2 DMAs/pair.

The batch pipeline is flattened across the 4 pairs (QK two batches ahead,
exp one ahead of PV) so no engine drains at pair boundaries.

Host side (O(T*D) only): input packing, the global-token rank-1 slot
(e0 = exp(q . k0)), the global query row (token 0 attends to all keys),
and the final division by the denominator row.
"""

import numpy as np
import ml_dtypes

N, H, T, D = 2, 16, 4000, 64
BLOCK = 128
TP = 4096            # padded token count (32 blocks)
W = 32               # number of 128-blocks
NCORES = 8
PAIRS = N * H        # 32
PPC = PAIRS // NCORES  # pairs per core
NEG = -30000.0
SCALE = 1.0 / np.sqrt(np.float32(D))
BQ = 3               # slabs (k-blocks) per QK/exp batch

_prog_cache = {}


def _qlo(j):
    return min(max(j - 1, 0), W - 3)


def _batches():
    out, j = [], 0
    while j < W:
        out.append(list(range(j, min(j + BQ, W))))
        j += BQ
    return out


def _build_program():
    if "nc" in _prog_cache:
        return _prog_cache["nc"]

    import concourse.bacc as bacc
    import concourse.mybir as mybir
    from concourse import tile

    dt = mybir.dt
    EXP = mybir.ActivationFunctionType.Exp

    nc = bacc.Bacc("TRN2", target_bir_lowering=False, debug=False,
                   num_devices=NCORES)
    qts_d = nc.dram_tensor("qts", [PPC, 65, TP], dt.bfloat16,
                           kind="ExternalInput").ap()
    kte_d = nc.dram_tensor("kte", [PPC, 65, TP], dt.bfloat16,
                           kind="ExternalInput").ap()
    vp_d = nc.dram_tensor("vp", [PPC, 128, W * 65], dt.bfloat16,
                          kind="ExternalInput").ap()
    out_d = nc.dram_tensor("out", [PPC, 65, TP], dt.float32,
                           kind="ExternalOutput").ap()

    pair_batches = _batches()          # per-pair batch list (slab indices)
    NPB = len(pair_batches)
    # global flattened batch list: (pair, slabs)
    gbatches = [(p, sl) for p in range(PPC) for sl in pair_batches]
    NB = len(gbatches)

    with tile.TileContext(nc) as tc:
        with (
            tc.tile_pool(name="qts", bufs=2) as qts_pool,
            tc.tile_pool(name="kte", bufs=2) as kte_pool,
            tc.tile_pool(name="vp", bufs=2) as vp_pool,
            tc.tile_pool(name="ex", bufs=3) as ex_pool,
            tc.tile_pool(name="small", bufs=1) as small_pool,
            tc.tile_pool(name="outp", bufs=2) as out_pool,
            tc.tile_pool(name="sc", bufs=2, space="PSUM") as sc_pool,
            tc.tile_pool(name="ctx", bufs=2, space="PSUM") as ctx_pool,
        ):
            def load_pair(p):
                # chunked so the first QK only waits on the head of the
                # stream (subtile deps), and K/Q arrive before V
                kte_t = kte_pool.tile([65, TP], dt.bfloat16, tag="kte",
                                      name=f"kte_{p}")
                qts_t = qts_pool.tile([65, TP], dt.bfloat16, tag="qts",
                                      name=f"qts_{p}")
                vp_t = vp_pool.tile([128, W * 65], dt.bfloat16, tag="vp",
                                    name=f"vp_{p}")
                nc.sync.dma_start(kte_t[:, 0:1024], kte_d[p, :, 0:1024])
                nc.sync.dma_start(qts_t[:, 0:1024], qts_d[p, :, 0:1024])
                nc.sync.dma_start(vp_t[:, 0:520], vp_d[p, :, 0:520])
                nc.sync.dma_start(kte_t[:, 1024:TP], kte_d[p, :, 1024:TP])
                nc.sync.dma_start(qts_t[:, 1024:TP], qts_d[p, :, 1024:TP])
                nc.sync.dma_start(vp_t[:, 520:W * 65], vp_d[p, :, 520:W * 65])
                return qts_t, kte_t, vp_t

            # PE warm-up: dense matmuls on memset data un-throttle the HAM
            # clock gate while the first pair's inputs stream in.
            warm_sb = small_pool.tile([128, 512], dt.bfloat16, tag="warm")
            nc.gpsimd.memset(warm_sb[:], 0.25)
            warm_ps = sc_pool.tile([128, 512], dt.float32, tag="sc",
                                   name="warm_ps")
            for r in range(7):
                nc.tensor.matmul(warm_ps[:], warm_sb[:, 0:128],
                                 warm_sb[:], start=True, stop=True)

            pending = {0: load_pair(0)}
            tiles = {}                  # pair -> (qts_t, kte_t, vp_t)
            outts = {}                  # pair -> out tile
            ex_tiles = {}               # global batch idx -> ex tile
            ctx_tiles = {}              # (pair, group) -> psum tile
            slab_gb = {}                # (pair, slab) -> global batch idx
            for gb, (p, sl) in enumerate(gbatches):
                for j in sl:
                    slab_gb[(p, j)] = gb

            def get_pair(p):
                if p not in tiles:
                    tiles[p] = pending.pop(p)
                    if p + 1 < PPC:
                        pending[p + 1] = load_pair(p + 1)
                    outts[p] = out_pool.tile([65, TP], dt.float32, tag="out",
                                             name=f"out_{p}")
                return tiles[p]

            def emit_qk(gb):
                p, sl = gbatches[gb]
                qts_t, kte_t, _ = get_pair(p)
                sc = sc_pool.tile([128, BQ * 512], dt.float32, tag="sc",
                                  name=f"sc_{p}_{gb}")
                for i, j in enumerate(sl):
                    c0 = _qlo(j) * 128
                    nc.tensor.matmul(
                        sc[:, i * 512:i * 512 + 384],
                        kte_t[:, j * 128:(j + 1) * 128],
                        qts_t[:, c0:c0 + 384],
                        start=True, stop=True)
                return sc

            def emit_exp(gb, sc):
                p, sl = gbatches[gb]
                nb = len(sl)
                ex = ex_pool.tile([128, BQ * 384], dt.bfloat16, tag="ex",
                                  name=f"ex_{p}_{gb}")
                nc.scalar.activation(
                    ex[:, 0:nb * 384].rearrange("p (b x) -> p b x", x=384),
                    sc[:].rearrange("p (b x) -> p b x", x=512)[:, 0:nb, 0:384],
                    EXP)
                ex_tiles[gb] = ex

            def emit_pv(gb):
                p, sl = gbatches[gb]
                _, _, vp_t = get_pair(p)
                outt = outts[p]
                for j in sl:
                    for w in (j - 1, j, j + 1):
                        if not (0 <= w < W):
                            continue
                        g, wi = w // 4, w % 4
                        key = (p, g)
                        if key not in ctx_tiles:
                            ctx_tiles[key] = ctx_pool.tile(
                                [65, 512], dt.float32, tag="ctx",
                                name=f"ct_{p}_{g}")
                        ct = ctx_tiles[key]
                        gsl = w - _qlo(j)
                        bb = slab_gb[(p, j)]
                        exm = ex_tiles[bb]
                        off = (j - gbatches[bb][1][0]) * 384 + gsl * 128
                        # window 4g opens its bank: start=True clears the
                        # whole bank's has_written before any other window
                        # in the bank has written (slab-major order)
                        st = (wi == 0) and (j == max(w - 1, 0))
                        sp = (j == min(w + 1, W - 1))
                        nc.tensor.matmul(
                            ct[:, wi * 128:(wi + 1) * 128],
                            vp_t[:, j * 65:(j + 1) * 65],
                            exm[:, off:off + 128],
                            start=st, stop=sp,
                            skip_group_check=True)
                    # close groups whose last contributor is slab j
                    for g in range(8):
                        if (p, g) in ctx_tiles and min(4 * g + 4, W - 1) == j:
                            ct = ctx_tiles.pop((p, g))
                            nc.vector.tensor_scalar_mul(
                                outt[:, g * 512:(g + 1) * 512], ct[:], 1.0)
                            if g == 3:
                                nc.sync.dma_start(
                                    out_d[p, :, 0:2048], outt[:, 0:2048])
                            elif g == 7:
                                nc.sync.dma_start(
                                    out_d[p, :, 2048:4096],
                                    outt[:, 2048:4096])

            # software pipeline over the flattened batch list: QK two
            # batches ahead, exp one ahead of the PV consumption; PV before
            # the next QK so the PE never queues behind an exp it doesn't
            # depend on.
            scs = {0: emit_qk(0), 1: emit_qk(1)}
            for gb in range(NB):
                emit_exp(gb, scs.pop(gb))
                if gb >= 1:
                    emit_pv(gb - 1)
                if gb + 2 < NB:
                    scs[gb + 2] = emit_qk(gb + 2)
            emit_pv(NB - 1)

    nc.compile()
    _prog_cache["nc"] = nc
    return nc


def _prep_core_inputs(q, k, v, mask):
    """q,k,v: (PAIRS, T, D) f32; mask: (N, T) f32.  Returns per-core input
    dicts (bf16 device layouts)."""
    bf16 = ml_dtypes.bfloat16
    mpair = np.repeat(mask, H, axis=0)              # (PAIRS, T)

    qt = np.zeros((PAIRS, 65, TP), np.float32)
    qt[:, :D, :T] = q.transpose(0, 2, 1) * SCALE
    qt[:, D, :] = 1.0

    kt = np.zeros((PAIRS, 65, TP), np.float32)
    kt[:, :D, :T] = k.transpose(0, 2, 1)
    kt[:, D, :T] = mpair
    kt[:, D, T:] = NEG
    kt[:, D, 0] = NEG                               # k0 served by global slot

    vp = np.zeros((PAIRS, TP, 65), np.float32)
    vp[:, :T, :D] = v
    vp[:, :, D] = 1.0
    vp = vp.reshape(PAIRS, W, 128, 65).transpose(0, 2, 1, 3) \
           .reshape(PAIRS, 128, W * 65)

    qt = qt.astype(bf16)
    kt = kt.astype(bf16)
    vp = vp.astype(bf16)
    in_maps = []
    for c in range(NCORES):
        s = slice(c * PPC, (c + 1) * PPC)
        in_maps.append({"qts": qt[s], "kte": kt[s], "vp": vp[s]})
    return in_maps


def _host_global(q, k, v, mask):
    """e0 (token-0 key slot, per query) and the global query row, in f32."""
    mpair = np.repeat(mask, H, axis=0)              # (PAIRS, T)
    k0 = k[:, 0, :]                                 # (PAIRS, D)
    s0 = np.einsum('ptd,pd->pt', q, k0) * SCALE + mpair[:, 0:1]
    e0 = np.exp(s0)                                 # (PAIRS, T)

    q0 = q[:, 0, :]                                 # (PAIRS, D)
    gs = np.einsum('pd,ptd->pt', q0, k) * SCALE + mpair
    gs -= gs.max(axis=1, keepdims=True)
    ge = np.exp(gs)
    grow = np.einsum('pt,ptd->pd', ge, v) / ge.sum(axis=1, keepdims=True)
    return e0, grow


def _unshard(results, e0, grow, v0):
    o = np.concatenate([r["out"] for r in results], axis=0)  # (PAIRS,65,TP)
    ctx = o[:, :D, :T]                              # (PAIRS, D, T)
    den = o[:, D, :T] + e0                          # (PAIRS, T)
    ctx = ctx + e0[:, None, :] * v0[:, :, None]
    out = (ctx / den[:, None, :]).transpose(0, 2, 1)  # (PAIRS, T, D)
    out[:, 0, :] = grow
    return np.ascontiguousarray(out.reshape(N, H, T, D), dtype=np.float32)


def _run(inputs, trace=False, tmpdir=None):
    from concourse.bass_utils import run_bass_kernel_spmd

    q = np.asarray(inputs["query_layer"], np.float32).reshape(PAIRS, T, D)
    k = np.asarray(inputs["key_layer"], np.float32).reshape(PAIRS, T, D)
    v = np.asarray(inputs["value_layer"], np.float32).reshape(PAIRS, T, D)
    mask = np.asarray(inputs["attention_mask"], np.float32).reshape(N, T)

    nc = _build_program()
    in_maps = _prep_core_inputs(q, k, v, mask)
    e0, grow = _host_global(q, k, v, mask)
    res = run_bass_kernel_spmd(nc, in_maps, list(range(NCORES)),
                               trace=trace, tmpdir=tmpdir)
    return _unshard(res.results, e0, grow, v[:, 0, :]), res


def kernel(query_layer, key_layer, value_layer, attention_mask):
    out, _ = _run({
        "query_layer": query_layer,
        "key_layer": key_layer,
        "value_layer": value_layer,
        "attention_mask": attention_mask,
    })
    return out


# revision 11
# speedup vs baseline: 1.9429x; 1.1189x over previous
"""Block-local self-attention (BLOCK=128, 3-block windows + global token) on 8
Trainium2 NeuronCores.

Sharding: batch*heads = 32 (n,h) pairs -> 4 pairs per core, no cross-core comms.

Per-core device kernel, per pair (all heavy O(T*window) work):
  - QK: for each k-block j (32), one matmul scoresT[k in j, q in blocks
    qlo..qlo+2] = K_j^T Q (stationary = K_j [65,128] incl. a mask row,
    moving = a contiguous [65,384] slice of the natural Q^T layout; the
    1/sqrt(d) scale is folded into Q on the host, the additive mask rides
    as a 65th contraction row).  3 slabs share one [128,1536] PSUM tile.
  - exp on ScalarE: one ACTIVATE per 3-slab batch, PSUM->SBUF bf16.
  - PV transposed: stationary = V'_j [128,65] ([V | ones] block; the ones
    column accumulates the softmax denominator), moving = 128-wide exp
    slices -> ctxT[d, q] accumulated in PSUM.  4 windows share one PSUM
    bank ([65,512]); window 4g's first matmul opens the bank with
    start=True (the whole-bank has_written clear happens before any other
    window touches the bank, and later windows' first writes land on
    cleared bits = overwrite), so no separate bank-clear is needed.
  - DVE copies each closed ctxT bank to an SBUF out tile; 2 DMAs/pair.

The batch pipeline is flattened across the 4 pairs (QK two batches ahead,
exp one ahead of PV) so no engine drains at pair boundaries.

Host side (O(T*D) only): input packing, the global-token rank-1 slot
(e0 = exp(q . k0)), the global query row (token 0 attends to all keys),
and the final division by the denominator row.
"""

import numpy as np
import ml_dtypes

N, H, T, D = 2, 16, 4000, 64
BLOCK = 128
TP = 4096            # padded token count (32 blocks)
W = 32               # number of 128-blocks
NCORES = 8
PAIRS = N * H        # 32
PPC = PAIRS // NCORES  # pairs per core
NEG = -30000.0
SCALE = 1.0 / np.sqrt(np.float32(D))
BQ = 3               # slabs (k-blocks) per QK/exp batch

_prog_cache = {}


def _qlo(j):
    return min(max(j - 1, 0), W - 3)


def _batches():
    out, j = [], 0
    while j < W:
        out.append(list(range(j, min(j + BQ, W))))
        j += BQ
    return out


def _build_program():
    if "nc" in _prog_cache:
        return _prog_cache["nc"]

    import concourse.bacc as bacc
    import concourse.mybir as mybir
    from concourse import tile

    dt = mybir.dt
    EXP = mybir.ActivationFunctionType.Exp

    nc = bacc.Bacc("TRN2", target_bir_lowering=False, debug=False,
                   num_devices=NCORES)
    qts_d = nc.dram_tensor("qts", [PPC, 65, TP], dt.bfloat16,
                           kind="ExternalInput").ap()
    kte_d = nc.dram_tensor("kte", [PPC, 65, TP], dt.bfloat16,
                           kind="ExternalInput").ap()
    vp_d = nc.dram_tensor("vp", [PPC, 128, W * 65], dt.bfloat16,
                          kind="ExternalInput").ap()
    out_d = nc.dram_tensor("out", [PPC, 65, TP], dt.bfloat16,
                           kind="ExternalOutput").ap()

    pair_batches = _batches()          # per-pair batch list (slab indices)
    NPB = len(pair_batches)
    # global flattened batch list: (pair, slabs)
    gbatches = [(p, sl) for p in range(PPC) for sl in pair_batches]
    NB = len(gbatches)

    with tile.TileContext(nc) as tc:
        with (
            tc.tile_pool(name="qts", bufs=2) as qts_pool,
            tc.tile_pool(name="kte", bufs=2) as kte_pool,
            tc.tile_pool(name="vp", bufs=2) as vp_pool,
            tc.tile_pool(name="ex", bufs=3) as ex_pool,
            tc.tile_pool(name="small", bufs=1) as small_pool,
            tc.tile_pool(name="outp", bufs=2) as out_pool,
            tc.tile_pool(name="sc", bufs=2, space="PSUM") as sc_pool,
            tc.tile_pool(name="ctx", bufs=2, space="PSUM") as ctx_pool,
        ):
            def load_pair(p):
                # chunked so the first QK only waits on the head of the
                # stream (subtile deps), and spread across the Sync and
                # GpSimd HWDGE rings (descriptor issue is ~900ns each; the
                # Scalar ring is reserved for the bottleneck ACT queue)
                kte_t = kte_pool.tile([65, TP], dt.bfloat16, tag="kte",
                                      name=f"kte_{p}")
                qts_t = qts_pool.tile([65, TP], dt.bfloat16, tag="qts",
                                      name=f"qts_{p}")
                vp_t = vp_pool.tile([128, W * 65], dt.bfloat16, tag="vp",
                                    name=f"vp_{p}")
                nc.sync.dma_start(kte_t[:, 0:512], kte_d[p, :, 0:512])
                nc.gpsimd.dma_start(qts_t[:, 0:512], qts_d[p, :, 0:512])
                nc.sync.dma_start(kte_t[:, 512:TP], kte_d[p, :, 512:TP])
                nc.gpsimd.dma_start(qts_t[:, 512:TP], qts_d[p, :, 512:TP])
                nc.sync.dma_start(vp_t[:], vp_d[p])
                return qts_t, kte_t, vp_t

            # PE warm-up: dense matmuls on memset data un-throttle the HAM
            # clock gate while the first pair's inputs stream in.
            warm_sb = small_pool.tile([128, 512], dt.bfloat16, tag="warm")
            nc.gpsimd.memset(warm_sb[:], 0.25)
            # preload the ACT exp table (~1.5us) during the DMA wait
            warm_ex = small_pool.tile([1, 1], dt.bfloat16, tag="wex")
            nc.scalar.activation(warm_ex[:], warm_sb[0:1, 0:1], EXP)
            warm_ps = sc_pool.tile([128, 512], dt.float32, tag="sc",
                                   name="warm_ps")
            for r in range(7):
                nc.tensor.matmul(warm_ps[:], warm_sb[:, 0:128],
                                 warm_sb[:], start=True, stop=True)

            pending = {0: load_pair(0)}
            tiles = {}                  # pair -> (qts_t, kte_t, vp_t)
            outts = {}                  # pair -> out tile
            ex_tiles = {}               # global batch idx -> ex tile
            ctx_tiles = {}              # (pair, group) -> psum tile
            slab_gb = {}                # (pair, slab) -> global batch idx
            for gb, (p, sl) in enumerate(gbatches):
                for j in sl:
                    slab_gb[(p, j)] = gb

            def get_pair(p):
                if p not in tiles:
                    tiles[p] = pending.pop(p)
                    if p + 1 < PPC:
                        pending[p + 1] = load_pair(p + 1)
                    outts[p] = out_pool.tile([65, TP], dt.bfloat16, tag="out",
                                             name=f"out_{p}")
                return tiles[p]

            def emit_qk(gb):
                p, sl = gbatches[gb]
                qts_t, kte_t, _ = get_pair(p)
                sc = sc_pool.tile([128, BQ * 512], dt.float32, tag="sc",
                                  name=f"sc_{p}_{gb}")
                for i, j in enumerate(sl):
                    c0 = _qlo(j) * 128
                    nc.tensor.matmul(
                        sc[:, i * 512:i * 512 + 384],
                        kte_t[:, j * 128:(j + 1) * 128],
                        qts_t[:, c0:c0 + 384],
                        start=True, stop=True)
                return sc

            def emit_exp(gb, sc):
                p, sl = gbatches[gb]
                nb = len(sl)
                ex = ex_pool.tile([128, BQ * 384], dt.bfloat16, tag="ex",
                                  name=f"ex_{p}_{gb}")
                nc.scalar.activation(
                    ex[:, 0:nb * 384].rearrange("p (b x) -> p b x", x=384),
                    sc[:].rearrange("p (b x) -> p b x", x=512)[:, 0:nb, 0:384],
                    EXP)
                ex_tiles[gb] = ex

            def emit_pv(gb):
                p, sl = gbatches[gb]
                _, _, vp_t = get_pair(p)
                outt = outts[p]
                for j in sl:
                    for w in (j - 1, j, j + 1):
                        if not (0 <= w < W):
                            continue
                        g, wi = w // 4, w % 4
                        key = (p, g)
                        if key not in ctx_tiles:
                            ctx_tiles[key] = ctx_pool.tile(
                                [65, 512], dt.float32, tag="ctx",
                                name=f"ct_{p}_{g}")
                        ct = ctx_tiles[key]
                        gsl = w - _qlo(j)
                        bb = slab_gb[(p, j)]
                        exm = ex_tiles[bb]
                        off = (j - gbatches[bb][1][0]) * 384 + gsl * 128
                        # window 4g opens its bank: start=True clears the
                        # whole bank's has_written before any other window
                        # in the bank has written (slab-major order)
                        st = (wi == 0) and (j == max(w - 1, 0))
                        sp = (j == min(w + 1, W - 1))
                        nc.tensor.matmul(
                            ct[:, wi * 128:(wi + 1) * 128],
                            vp_t[:, j * 65:(j + 1) * 65],
                            exm[:, off:off + 128],
                            start=st, stop=sp,
                            skip_group_check=True)
                    # close groups whose last contributor is slab j; ship
                    # every 2 groups on the Vector ring (DVE just wrote it)
                    for g in range(8):
                        if (p, g) in ctx_tiles and min(4 * g + 4, W - 1) == j:
                            ct = ctx_tiles.pop((p, g))
                            nc.vector.tensor_scalar_mul(
                                outt[:, g * 512:(g + 1) * 512], ct[:], 1.0)
                            if g % 2 == 1:
                                c0 = (g - 1) * 512
                                nc.sync.dma_start(
                                    out_d[p, :, c0:c0 + 1024],
                                    outt[:, c0:c0 + 1024])

            # software pipeline over the flattened batch list: QK two
            # batches ahead, exp one ahead of the PV consumption; PV before
            # the next QK so the PE never queues behind an exp it doesn't
            # depend on.
            scs = {0: emit_qk(0), 1: emit_qk(1)}
            for gb in range(NB):
                emit_exp(gb, scs.pop(gb))
                if gb >= 1:
                    emit_pv(gb - 1)
                if gb + 2 < NB:
                    scs[gb + 2] = emit_qk(gb + 2)
            emit_pv(NB - 1)

    nc.compile()
    _prog_cache["nc"] = nc
    return nc


def _prep_core_inputs(q, k, v, mask):
    """q,k,v: (PAIRS, T, D) f32; mask: (N, T) f32.  Returns per-core input
    dicts (bf16 device layouts)."""
    bf16 = ml_dtypes.bfloat16
    mpair = np.repeat(mask, H, axis=0)              # (PAIRS, T)

    qt = np.zeros((PAIRS, 65, TP), np.float32)
    qt[:, :D, :T] = q.transpose(0, 2, 1) * SCALE
    qt[:, D, :] = 1.0

    kt = np.zeros((PAIRS, 65, TP), np.float32)
    kt[:, :D, :T] = k.transpose(0, 2, 1)
    kt[:, D, :T] = mpair
    kt[:, D, T:] = NEG
    kt[:, D, 0] = NEG                               # k0 served by global slot

    vp = np.zeros((PAIRS, TP, 65), np.float32)
    vp[:, :T, :D] = v
    vp[:, :, D] = 1.0
    vp = vp.reshape(PAIRS, W, 128, 65).transpose(0, 2, 1, 3) \
           .reshape(PAIRS, 128, W * 65)

    qt = qt.astype(bf16)
    kt = kt.astype(bf16)
    vp = vp.astype(bf16)
    in_maps = []
    for c in range(NCORES):
        s = slice(c * PPC, (c + 1) * PPC)
        in_maps.append({"qts": qt[s], "kte": kt[s], "vp": vp[s]})
    return in_maps


def _host_global(q, k, v, mask):
    """e0 (token-0 key slot, per query) and the global query row, in f32."""
    mpair = np.repeat(mask, H, axis=0)              # (PAIRS, T)
    k0 = k[:, 0, :]                                 # (PAIRS, D)
    s0 = np.einsum('ptd,pd->pt', q, k0) * SCALE + mpair[:, 0:1]
    e0 = np.exp(s0)                                 # (PAIRS, T)

    q0 = q[:, 0, :]                                 # (PAIRS, D)
    gs = np.einsum('pd,ptd->pt', q0, k) * SCALE + mpair
    gs -= gs.max(axis=1, keepdims=True)
    ge = np.exp(gs)
    grow = np.einsum('pt,ptd->pd', ge, v) / ge.sum(axis=1, keepdims=True)
    return e0, grow


def _unshard(results, e0, grow, v0):
    o = np.concatenate([r["out"] for r in results], axis=0) \
          .astype(np.float32)                       # (PAIRS, 65, TP)
    ctx = o[:, :D, :T]                              # (PAIRS, D, T)
    den = o[:, D, :T] + e0                          # (PAIRS, T)
    ctx = ctx + e0[:, None, :] * v0[:, :, None]
    out = (ctx / den[:, None, :]).transpose(0, 2, 1)  # (PAIRS, T, D)
    out[:, 0, :] = grow
    return np.ascontiguousarray(out.reshape(N, H, T, D), dtype=np.float32)


def _run(inputs, trace=False, tmpdir=None):
    from concourse.bass_utils import run_bass_kernel_spmd

    q = np.asarray(inputs["query_layer"], np.float32).reshape(PAIRS, T, D)
    k = np.asarray(inputs["key_layer"], np.float32).reshape(PAIRS, T, D)
    v = np.asarray(inputs["value_layer"], np.float32).reshape(PAIRS, T, D)
    mask = np.asarray(inputs["attention_mask"], np.float32).reshape(N, T)

    nc = _build_program()
    in_maps = _prep_core_inputs(q, k, v, mask)
    e0, grow = _host_global(q, k, v, mask)
    res = run_bass_kernel_spmd(nc, in_maps, list(range(NCORES)),
                               trace=trace, tmpdir=tmpdir)
    return _unshard(res.results, e0, grow, v[:, 0, :]), res


def kernel(query_layer, key_layer, value_layer, attention_mask):
    out, _ = _run({
        "query_layer": query_layer,
        "key_layer": key_layer,
        "value_layer": value_layer,
        "attention_mask": attention_mask,
    })
    return out
